# revision 24
# baseline (speedup 1.0000x reference)
"""Block-FFT circulant matmul (BlockFFTDirectPrior) as a Trainium2 Bass kernel.

Math: out = ifft( einsum('bjf,ijf->bif', fft(x_blocks), conj(W_full)) ).real
with 64x64 blocks of size 256, batch 2048.

All matmul-based (no FFT butterflies):
  stage 1: per input block j, spectrum = x_j @ R            (DFT as matmul)
  stage 2: per frequency slot s, mix blocks j -> i with a 128x128 real
           matrix G_s built from W_real/W_imag (re/im packed)
  stage 3: per output block i, time = spectrum_i @ Rinv     (IDFT as matmul)

v5: two 64KB/partition ping-pong buffers (A holds x -> stage-1 output
in place -> stage-2 output; B2 holds the transposed spectra).  The two
inter-stage permutes per pass are split between the DMA xbar (which
monopolizes the fabric, so it only runs in wire-quiet windows: perm1
pass-1 chunked under stage-1 pass-1, perm2 pass-0 paired with a
concurrent PE-transpose batch) and tensor-engine is_transpose matmuls
(perm1 pass-0 interleaved in stage-1 pass-1's warm matmul stream,
perm2 pass-1 batched while the xbar runs).  Stage 2 is merged across
both passes: one LDWEIGHTS + one N=256 matmul per frequency slot,
halving its time and keeping the PE busy-fraction high enough that the
HAM clock stays at 2.4 GHz.

Sharding: data-parallel over batch across 8 NeuronCores (256 rows each),
2 passes of 128 rows per core.  All matmul operands are bf16.
"""

import numpy as np
import ml_dtypes

import concourse.bass as bass
import concourse.mybir as mybir
from concourse import bacc
from concourse.tile import TileContext
from concourse.bass_utils import run_bass_kernel_spmd

B, KIN, KOUT, BLOCK = 2048, 64, 64, 256
NCORES = 8
BC = B // NCORES            # 256 batch rows per core
NPASS = 2
PB = BC // NPASS            # 128 batch rows per pass

F32 = mybir.dt.float32
BF16 = mybir.dt.bfloat16
NPBF16 = ml_dtypes.bfloat16

_NC_CACHE = {}


def _build_consts():
    """DFT / inverse-DFT matrices, bf16, kernel layouts."""
    t = np.arange(BLOCK)
    f = np.arange(128)
    ang = 2.0 * np.pi * np.outer(t, f) / BLOCK          # [t, f]
    RA = np.cos(ang)                                    # re f=0..127
    RB = -np.sin(ang)                                   # im f=1..127
    RB[:, 0] = np.cos(np.pi * t)                        # re f=128 in col 0
    R = np.zeros((128, 4, 128), dtype=NPBF16)           # [t128, (h,kt), s]
    for h in range(2):
        M = RA if h == 0 else RB
        for kt in range(2):
            R[:, h * 2 + kt, :] = M[kt * 128:(kt + 1) * 128, :].astype(NPBF16)

    s = np.arange(128)
    tp = np.arange(BLOCK)
    angi = 2.0 * np.pi * np.outer(s, tp) / BLOCK        # [s, t']
    w = np.full((128, 1), 2.0 / BLOCK)
    w[0] = 1.0 / BLOCK
    RiA = w * np.cos(angi)
    RiB = -(2.0 / BLOCK) * np.sin(angi)
    RiB[0, :] = (1.0 / BLOCK) * np.cos(np.pi * tp)      # Nyquist (real) term
    Ri = np.concatenate([RiA, RiB], axis=1).astype(NPBF16)  # [128, 512]
    return R, Ri


def _build_g(Wr, Wi):
    """Stage-2 mixing matrices, layout [k=(h*64+j), s, m=(re_i|im_i)], bf16."""
    G = np.zeros((128, 128, 128), dtype=np.float32)     # [s, k, m]
    G[0, :64, :64] = Wr[:, :, 0].T
    G[0, 64:, 64:] = Wr[:, :, 128].T
    WrT = np.transpose(Wr, (2, 1, 0))                   # [f, j, i]
    WiT = np.transpose(Wi, (2, 1, 0))
    G[1:, :64, :64] = WrT[1:128]
    G[1:, :64, 64:] = -WiT[1:128]
    G[1:, 64:, :64] = WiT[1:128]
    G[1:, 64:, 64:] = WrT[1:128]
    return np.ascontiguousarray(G.transpose(1, 0, 2)).astype(NPBF16)


def _build_nc():
    nc = bacc.Bacc("TRN2", target_bir_lowering=False, debug=False)
    # xP layout [pass, tl(128), b(128), kt(2), j(64)]
    xP = nc.dram_tensor("xP", [NPASS, 128, PB, 2, KIN], BF16,
                        kind="ExternalInput")
    Gt = nc.dram_tensor("G", [128, 128, 128], BF16, kind="ExternalInput")
    Rt = nc.dram_tensor("R", [128, 4 * 128], BF16, kind="ExternalInput")
    Rit = nc.dram_tensor("Ri", [128, 512], BF16, kind="ExternalInput")
    It = nc.dram_tensor("Iden", [128, 128], BF16, kind="ExternalInput")
    # Y layout [pass, t'-chunk(2), i-oct(8), t'(128), (u2, b, i4)];
    # the host untangles this back to [b, i, t'] for free.
    Y = nc.dram_tensor("Y", [NPASS, 2, 8, 128, 8 * PB], BF16,
                       kind="ExternalOutput")

    nck = [0]

    def copy_eng():
        nck[0] += 1
        return nc.vector.tensor_copy if nck[0] % 2 == 0 else nc.scalar.copy

    with TileContext(nc) as tc:
        with (
            tc.tile_pool(name="const", bufs=1) as cpool,
            tc.tile_pool(name="big", bufs=1) as bigpool,
            tc.tile_pool(name="work", bufs=4) as wpool,
            tc.tile_pool(name="ps", bufs=3, space="PSUM") as pspool,
            tc.tile_pool(name="pt", bufs=2, space="PSUM") as ptpool,
        ):
            # constants: R/Ri/Iden on scalar queue, G on gpsimd
            Rsb = cpool.tile([128, 4 * 128], BF16)
            nc.scalar.dma_start(Rsb[:, :], Rt.ap()[:, :])
            Risb = cpool.tile([128, 512], BF16)
            nc.scalar.dma_start(Risb[:, :], Rit.ap()[:, :])
            Iden = cpool.tile([128, 128], BF16)
            nc.scalar.dma_start(Iden[:, :], It.ap()[:, :])

            # two 64KB ping-pong buffers
            # A:  xk(p0)|xk(p1) -> out1(p0)|out1(p1) in place -> O2(p0)|O2(p1)
            # B2: X2(p0)|X2(p1) -> T2(p0)|T2(p1)
            A = bigpool.tile([128, 2 * 128 * 128], BF16, tag="A", name="A")
            B2 = bigpool.tile([128, 2 * 128 * 128], BF16, tag="B2", name="B2")
            HALF = 128 * 128
            # views
            Axk = A.rearrange("p (pa b kt j) -> p pa b kt j", pa=2, kt=2, j=KIN)
            Ao1 = A.rearrange("p (pa b hj) -> p pa b hj", pa=2, hj=128)
            AO2 = A.rearrange("p (pa b s) -> p pa b s", pa=2, s=128)
            B2v = B2.rearrange("p (pa b s) -> p pa b s", pa=2, s=128)

            def load_xk(p, eng):
                for q in range(4):
                    eng.dma_start(
                        Axk[:, p, q * 32:(q + 1) * 32, :, :],
                        xP.ap()[p, :, q * 32:(q + 1) * 32],
                    )

            # G load on gpsimd, s-ordered so stage 2 can consume
            # progressively (col-chunk c covers s = 16c..16c+16).
            def load_g():
                Gsb = cpool.tile([128, 128 * 128], BF16)
                for q in range(8):
                    nc.gpsimd.dma_start(
                        Gsb[:, q * 2048:(q + 1) * 2048],
                        Gt.ap()[:, q * 16:(q + 1) * 16],
                    )
                return Gsb

            # PE-transpose permute steps: src [p, b, c128] -> dst [c128, b, p]
            # 8 transposes + 1 evict per step (one PSUM bank, bf16).
            def perm_steps(src_v, dst_v):
                def step(b8):
                    pt = ptpool.tile([128, 1024], BF16, tag="pt")
                    for o in range(8):
                        b = b8 * 8 + o
                        nc.tensor.transpose(
                            pt[:, o * 128:(o + 1) * 128],
                            src_v[:, b, :], Iden[:, :],
                        )
                    copy_eng()(
                        dst_v[:, b8 * 8:(b8 + 1) * 8, :],
                        pt.rearrange("p (b c) -> p b c", b=8),
                    )
                return [lambda b8=b8: step(b8) for b8 in range(16)]

            def stage1(p, inter=(), xbar_perm1=False):
                """DFT pass p, in place in A's half p.  Optionally fires
                perm1(p) xbar chunks (to B2) as b-ranges complete."""
                it = iter(inter)
                for g2 in range(8):             # 16-batch groups
                    pss = []
                    for h in range(2):
                        ps = pspool.tile([128, 1024], F32, tag="ps")
                        for q in range(2):
                            g = g2 * 2 + q      # 8-batch chunk
                            nc.tensor.matmul(
                                ps[:, q * 512:(q + 1) * 512],
                                Rsb[:, (h * 2) * 128:(h * 2 + 1) * 128],
                                Axk[:, p, g * 8:(g + 1) * 8, 0, :],
                                start=True, stop=False,
                            )
                            nc.tensor.matmul(
                                ps[:, q * 512:(q + 1) * 512],
                                Rsb[:, (h * 2 + 1) * 128:(h * 2 + 2) * 128],
                                Axk[:, p, g * 8:(g + 1) * 8, 1, :],
                                start=False, stop=True,
                            )
                        pss.append(ps)
                    # both h read the window before the copies overwrite it
                    for h in range(2):
                        # ps [s, (b16, j64)] -> out1 [s, b, h*64+j]
                        copy_eng()(
                            Ao1[:, p, g2 * 16:(g2 + 1) * 16,
                                h * 64:(h + 1) * 64],
                            pss[h].rearrange("p (b j) -> p b j", b=16),
                        )
                    for fn in it:               # interleaved perm steps
                        fn()
                        break
                    for fn in it:
                        fn()
                        break
                    if xbar_perm1 and g2 < 6:
                        nc.sync.dma_start(
                            B2v[:, p, g2 * 16:(g2 + 1) * 16, :],
                            A[:, p * HALF + g2 * 2048:p * HALF
                              + (g2 + 1) * 2048],
                            transpose=True,
                        )

            def stage2a(Gsb, pa, sg0, sg1):
                """Mixing for one pass only, slots s in [8*sg0, 8*sg1) —
                used to fill the pipeline while the other pass's perm1
                finishes (keeps the HAM clock warm)."""
                X2 = B2.rearrange("p (pa b s) -> p pa b s", pa=2, s=128)
                for sg in range(sg0, sg1):      # 8 slots per PSUM tile
                    ps = pspool.tile([128, 1024], F32, tag="ps")
                    for u in range(8):
                        s = sg * 8 + u
                        nc.tensor.matmul(
                            ps[:, u * 128:(u + 1) * 128],
                            Gsb[:, s * 128:(s + 1) * 128],
                            X2[:, pa, :, s],
                            start=True, stop=True,
                        )
                    # ps [m, (s8, b)] -> O2 [m, pa, b, s]
                    copy_eng()(
                        AO2[:, pa, :, sg * 8:(sg + 1) * 8],
                        ps.rearrange("p (s b) -> p b s", s=8),
                    )

            def stage2m(Gsb):
                """Mixing, both passes per slot.  Each matmul streams the
                s-PAIR (N=512, half of it discarded) purely to keep the PE
                duty cycle high enough that the HAM clock stays at 2.4 GHz;
                the weight-switch overhead absorbs the extra columns."""
                # s-pair as the OUTERMOST free dim so the kept half of each
                # matmul's output is a contiguous 256-col PSUM block (the
                # evict's stride then sits on the cheap write side)
                X2s = B2.rearrange("p (pa b s) -> p s pa b", pa=2, s=128)
                for t in range(64):             # 2 slots per PSUM tile
                    s0 = 2 * t
                    ps = pspool.tile([128, 1024], F32, tag="ps")
                    rhs = X2s[:, s0:s0 + 2, :, :]   # free (2, pa, b) = 512
                    nc.tensor.matmul(
                        ps[:, 0:512],
                        Gsb[:, s0 * 128:(s0 + 1) * 128],
                        rhs, start=True, stop=True,
                    )
                    nc.tensor.matmul(
                        ps[:, 512:1024],
                        Gsb[:, (s0 + 1) * 128:(s0 + 2) * 128],
                        rhs, start=True, stop=True,
                    )
                    # keep sl=0 of the first MM, sl=1 of the second
                    nc.vector.tensor_copy(
                        AO2[:, :, :, s0],
                        ps[:, 0:256].rearrange("p (pa b) -> p pa b", pa=2),
                    )
                    nc.scalar.copy(
                        AO2[:, :, :, s0 + 1],
                        ps[:, 768:1024].rearrange("p (pa b) -> p pa b", pa=2),
                    )

            def stage3(p):
                """IDFT with Ri as the stationary: N=512 matmuls covering 4
                output blocks each (rhs = T2v[:, :, i:i+4]), one 2KB PSUM
                zero-region per matmul so A/B accumulation pairs are legal.
                Output layout [t', (u2, b, i4)]; the host reorders."""
                T2v = B2v[:, p]                 # [s, b, m]
                for c in range(2):              # t' chunk
                    for e in range(8):          # 8 output blocks i per tile
                        ps = pspool.tile([128, 1024], F32, tag="ps",
                                         name="ps3")
                        for h in range(2):      # RiA then RiB (accumulate)
                            off = c * 128 if h == 0 else 256 + c * 128
                            for u in range(2):
                                i0 = h * 64 + e * 8 + u * 4
                                nc.tensor.matmul(
                                    ps[:, u * 512:(u + 1) * 512],
                                    Risb[:, off:off + 128],
                                    T2v[:, :, i0:i0 + 4],
                                    start=(h == 0), stop=(h == 1),
                                )
                        yt = wpool.tile([128, 1024], BF16, tag="yt",
                                        name="yt")
                        copy_eng()(yt[:, :], ps[:, :])
                        nc.gpsimd.dma_start(Y.ap()[p, c, e], yt[:, :])

            # Emission.
            # PE:   S1p0 | S1p1+T1p0 | S2a(p0) | S2a(p1) | S2m |
            #       T2p1-batch | S3p1 | S3p0
            # xbar: T1p1 chunks (under S1p1/S2a) ... T2p0 (under T2p1/S3p1)
            load_xk(0, nc.sync)
            load_xk(1, nc.sync)
            Gsb = load_g()
            stage1(0)
            # perm1(p0): PE transposes interleaved; perm1(p1): xbar chunks
            # for b<96 + a PE coda for b 96..127 (keeps the PE fed through
            # the xbar tail so the HAM clock stays warm)
            stage1(1, inter=perm_steps(Ao1[:, 0], B2v[:, 0]), xbar_perm1=True)
            for fn in perm_steps(Ao1[:, 1], B2v[:, 1])[12:16]:
                fn()
            stage2m(Gsb)
            # perm2(p0): xbar for b<96 + PE coda; perm2(p1) on the PE
            for c in range(3):
                nc.sync.dma_start(
                    B2v[:, 0, c * 32:(c + 1) * 32, :],
                    A[:, c * 4096:(c + 1) * 4096], transpose=True,
                )
            for fn in perm_steps(AO2[:, 1], B2v[:, 1]):
                fn()
            for fn in perm_steps(AO2[:, 0], B2v[:, 0])[12:16]:
                fn()
            stage3(1)
            stage3(0)
    nc.compile()
    return nc


def _get_nc():
    if "nc" not in _NC_CACHE:
        _NC_CACHE["nc"] = _build_nc()
    return _NC_CACHE["nc"]


def run(x, W_real, W_imag, trace=False):
    x = np.asarray(x, dtype=np.float32)
    Wr = np.asarray(W_real, dtype=np.float32)
    Wi = np.asarray(W_imag, dtype=np.float32)

    nc = _get_nc()
    R, Ri = _build_consts()
    G = _build_g(Wr, Wi)
    Rflat = np.ascontiguousarray(R.reshape(128, 512))
    Iden = np.eye(128, dtype=NPBF16)

    in_maps = []
    for c in range(NCORES):
        xc = x[c * BC:(c + 1) * BC]                       # [256, 16384]
        # -> [pass, tl, b, kt, j]: x[pass*128+b, j, kt*128+tl]
        xcp = xc.reshape(NPASS, PB, KIN, 2, 128).transpose(0, 4, 1, 3, 2)
        in_maps.append({
            "xP": np.ascontiguousarray(xcp).astype(NPBF16),
            "G": G, "R": Rflat, "Ri": Ri, "Iden": Iden,
        })
    res = run_bass_kernel_spmd(
        nc, in_maps, core_ids=list(range(NCORES)), trace=trace
    )
    outs = []
    for r in res.results:
        yc = r["Y"].astype(np.float32)        # [pass, c, e, t', (u, b, i4)]
        yc = yc.reshape(NPASS, 2, 8, 128, 2, PB, 4)
        # i = e*8 + u*4 + i4; col = i*256 + c*128 + t
        yc = yc.transpose(0, 5, 2, 4, 6, 1, 3)  # [pass, b, e, u, i4, c, t]
        outs.append(yc.reshape(BC, KOUT * BLOCK))
    out = np.concatenate(outs, axis=0)
    return np.ascontiguousarray(out), res


def kernel(x, W_real, W_imag):
    out, _ = run(x, W_real, W_imag)
    return out


# revision 25
# speedup vs baseline: 1.3837x; 1.3837x over previous
"""Block-FFT circulant matmul (BlockFFTDirectPrior) as a Trainium2 Bass kernel.

Math: out = ifft( einsum('bjf,ijf->bif', fft(x_blocks), conj(W_full)) ).real
with 64x64 blocks of size 256, batch 2048.

All matmul-based (no FFT butterflies):
  stage 1: per input block j, spectrum = x_j @ R            (DFT as matmul)
  stage 2: per frequency slot s, mix blocks j -> i with a 128x128 real
           matrix G_s built from W_real/W_imag (re/im packed)
  stage 3: per output block i, time = spectrum_i @ Rinv     (IDFT as matmul)

v5: two 64KB/partition ping-pong buffers (A holds x -> stage-1 output
in place -> stage-2 output; B2 holds the transposed spectra).  The two
inter-stage permutes per pass are split between the DMA xbar (which
monopolizes the fabric, so it only runs in wire-quiet windows: perm1
pass-1 chunked under stage-1 pass-1, perm2 pass-0 paired with a
concurrent PE-transpose batch) and tensor-engine is_transpose matmuls
(perm1 pass-0 interleaved in stage-1 pass-1's warm matmul stream,
perm2 pass-1 batched while the xbar runs).  Stage 2 is merged across
both passes: one LDWEIGHTS + one N=256 matmul per frequency slot,
halving its time and keeping the PE busy-fraction high enough that the
HAM clock stays at 2.4 GHz.

Sharding: data-parallel over batch across 8 NeuronCores (256 rows each),
2 passes of 128 rows per core.  All matmul operands are bf16.
"""

import numpy as np
import ml_dtypes

import concourse.bass as bass
import concourse.mybir as mybir
from concourse import bacc
from concourse.tile import TileContext
from concourse.bass_utils import run_bass_kernel_spmd

B, KIN, KOUT, BLOCK = 2048, 64, 64, 256
NCORES = 8
BC = B // NCORES            # 256 batch rows per core
NPASS = 2
PB = BC // NPASS            # 128 batch rows per pass

F32 = mybir.dt.float32
BF16 = mybir.dt.bfloat16
NPBF16 = ml_dtypes.bfloat16

_NC_CACHE = {}


def _build_consts():
    """DFT / inverse-DFT matrices, bf16, kernel layouts."""
    t = np.arange(BLOCK)
    f = np.arange(128)
    ang = 2.0 * np.pi * np.outer(t, f) / BLOCK          # [t, f]
    RA = np.cos(ang)                                    # re f=0..127
    RB = -np.sin(ang)                                   # im f=1..127
    RB[:, 0] = np.cos(np.pi * t)                        # re f=128 in col 0
    R = np.zeros((128, 4, 128), dtype=NPBF16)           # [t128, (h,kt), s]
    for h in range(2):
        M = RA if h == 0 else RB
        for kt in range(2):
            R[:, h * 2 + kt, :] = M[kt * 128:(kt + 1) * 128, :].astype(NPBF16)

    s = np.arange(128)
    tp = np.arange(BLOCK)
    angi = 2.0 * np.pi * np.outer(s, tp) / BLOCK        # [s, t']
    w = np.full((128, 1), 2.0 / BLOCK)
    w[0] = 1.0 / BLOCK
    RiA = w * np.cos(angi)
    RiB = -(2.0 / BLOCK) * np.sin(angi)
    RiB[0, :] = (1.0 / BLOCK) * np.cos(np.pi * tp)      # Nyquist (real) term
    Ri = np.concatenate([RiA, RiB], axis=1).astype(NPBF16)  # [128, 512]
    return R, Ri


def _build_g(Wr, Wi):
    """Stage-2 mixing matrices, layout [k=(h*64+j), s, m=(re_i|im_i)], bf16."""
    G = np.zeros((128, 128, 128), dtype=np.float32)     # [s, k, m]
    G[0, :64, :64] = Wr[:, :, 0].T
    G[0, 64:, 64:] = Wr[:, :, 128].T
    WrT = np.transpose(Wr, (2, 1, 0))                   # [f, j, i]
    WiT = np.transpose(Wi, (2, 1, 0))
    G[1:, :64, :64] = WrT[1:128]
    G[1:, :64, 64:] = -WiT[1:128]
    G[1:, 64:, :64] = WiT[1:128]
    G[1:, 64:, 64:] = WrT[1:128]
    return np.ascontiguousarray(G.transpose(1, 0, 2)).astype(NPBF16)


def _build_nc():
    nc = bacc.Bacc("TRN2", target_bir_lowering=False, debug=False)
    # xP layout [pass, tl(128), b(128), kt(2), j(64)]
    xP = nc.dram_tensor("xP", [NPASS, 128, PB, 2, KIN], BF16,
                        kind="ExternalInput")
    Gt = nc.dram_tensor("G", [128, 128, 128], BF16, kind="ExternalInput")
    Rt = nc.dram_tensor("R", [128, 4 * 128], BF16, kind="ExternalInput")
    Rit = nc.dram_tensor("Ri", [128, 512], BF16, kind="ExternalInput")
    It = nc.dram_tensor("Iden", [128, 128], BF16, kind="ExternalInput")
    # Y layout [pass, t'-chunk(2), i-oct(8), t'(128), (u2, b, i4)];
    # the host untangles this back to [b, i, t'] for free.
    Y = nc.dram_tensor("Y", [NPASS, 2, 8, 128, 8 * PB], BF16,
                       kind="ExternalOutput")

    nck = [0]

    def copy_eng():
        nck[0] += 1
        return nc.vector.tensor_copy if nck[0] % 2 == 0 else nc.scalar.copy

    with TileContext(nc) as tc:
        with (
            tc.tile_pool(name="const", bufs=1) as cpool,
            tc.tile_pool(name="big", bufs=1) as bigpool,
            tc.tile_pool(name="work", bufs=4) as wpool,
            tc.tile_pool(name="ps", bufs=3, space="PSUM") as pspool,
            tc.tile_pool(name="pt", bufs=2, space="PSUM") as ptpool,
        ):
            # constants: R/Ri/Iden on scalar queue, G on gpsimd
            Rsb = cpool.tile([128, 4 * 128], BF16)
            nc.scalar.dma_start(Rsb[:, :], Rt.ap()[:, :])
            Risb = cpool.tile([128, 512], BF16)
            nc.scalar.dma_start(Risb[:, :], Rit.ap()[:, :])
            Iden = cpool.tile([128, 128], BF16)
            nc.scalar.dma_start(Iden[:, :], It.ap()[:, :])

            # two 64KB ping-pong buffers
            # A:  xk(p0)|xk(p1) -> out1(p0)|out1(p1) in place -> O2(p0)|O2(p1)
            # B2: X2(p0)|X2(p1) -> T2(p0)|T2(p1)
            A = bigpool.tile([128, 2 * 128 * 128], BF16, tag="A", name="A")
            B2 = bigpool.tile([128, 2 * 128 * 128], BF16, tag="B2", name="B2")
            HALF = 128 * 128
            # views
            Axk = A.rearrange("p (pa b kt j) -> p pa b kt j", pa=2, kt=2, j=KIN)
            Ao1 = A.rearrange("p (pa b hj) -> p pa b hj", pa=2, hj=128)
            AO2 = A.rearrange("p (pa b s) -> p pa b s", pa=2, s=128)
            B2v = B2.rearrange("p (pa b s) -> p pa b s", pa=2, s=128)

            def load_xk(p, eng):
                for q in range(4):
                    eng.dma_start(
                        Axk[:, p, q * 32:(q + 1) * 32, :, :],
                        xP.ap()[p, :, q * 32:(q + 1) * 32],
                    )

            # G load on gpsimd, s-ordered so stage 2 can consume
            # progressively (col-chunk c covers s = 16c..16c+16).
            def load_g():
                Gsb = cpool.tile([128, 128 * 128], BF16)
                for q in range(8):
                    nc.gpsimd.dma_start(
                        Gsb[:, q * 2048:(q + 1) * 2048],
                        Gt.ap()[:, q * 16:(q + 1) * 16],
                    )
                return Gsb

            # PE-transpose permute steps: src [p, b, c128] -> dst [c128, b, p]
            # 8 transposes + 1 evict per step (one PSUM bank, bf16).
            def perm_steps(src_v, dst_v):
                def step(b8):
                    pt = ptpool.tile([128, 1024], BF16, tag="pt")
                    for o in range(8):
                        b = b8 * 8 + o
                        nc.tensor.transpose(
                            pt[:, o * 128:(o + 1) * 128],
                            src_v[:, b, :], Iden[:, :],
                        )
                    copy_eng()(
                        dst_v[:, b8 * 8:(b8 + 1) * 8, :],
                        pt.rearrange("p (b c) -> p b c", b=8),
                    )
                return [lambda b8=b8: step(b8) for b8 in range(16)]

            def stage1(p, inter=(), xbar_perm1=False):
                """DFT pass p, in place in A's half p.  Optionally fires
                perm1(p) xbar chunks (to B2) as b-ranges complete."""
                it = iter(inter)
                for g2 in range(8):             # 16-batch groups
                    pss = []
                    for h in range(2):
                        ps = pspool.tile([128, 1024], F32, tag="ps")
                        for q in range(2):
                            g = g2 * 2 + q      # 8-batch chunk
                            nc.tensor.matmul(
                                ps[:, q * 512:(q + 1) * 512],
                                Rsb[:, (h * 2) * 128:(h * 2 + 1) * 128],
                                Axk[:, p, g * 8:(g + 1) * 8, 0, :],
                                start=True, stop=False,
                            )
                            nc.tensor.matmul(
                                ps[:, q * 512:(q + 1) * 512],
                                Rsb[:, (h * 2 + 1) * 128:(h * 2 + 2) * 128],
                                Axk[:, p, g * 8:(g + 1) * 8, 1, :],
                                start=False, stop=True,
                            )
                        pss.append(ps)
                    # both h read the window before the copies overwrite it
                    for h in range(2):
                        # ps [s, (b16, j64)] -> out1 [s, b, h*64+j]
                        copy_eng()(
                            Ao1[:, p, g2 * 16:(g2 + 1) * 16,
                                h * 64:(h + 1) * 64],
                            pss[h].rearrange("p (b j) -> p b j", b=16),
                        )
                    for fn in it:               # interleaved perm steps
                        fn()
                        break
                    for fn in it:
                        fn()
                        break
                    if xbar_perm1 and g2 < 6:
                        nc.sync.dma_start(
                            B2v[:, p, g2 * 16:(g2 + 1) * 16, :],
                            A[:, p * HALF + g2 * 2048:p * HALF
                              + (g2 + 1) * 2048],
                            transpose=True,
                        )

            def stage2a(Gsb, pa, sg0, sg1):
                """Mixing for one pass only, slots s in [8*sg0, 8*sg1) —
                used to fill the pipeline while the other pass's perm1
                finishes (keeps the HAM clock warm)."""
                X2 = B2.rearrange("p (pa b s) -> p pa b s", pa=2, s=128)
                for sg in range(sg0, sg1):      # 8 slots per PSUM tile
                    ps = pspool.tile([128, 1024], F32, tag="ps")
                    for u in range(8):
                        s = sg * 8 + u
                        nc.tensor.matmul(
                            ps[:, u * 128:(u + 1) * 128],
                            Gsb[:, s * 128:(s + 1) * 128],
                            X2[:, pa, :, s],
                            start=True, stop=True,
                        )
                    # ps [m, (s8, b)] -> O2 [m, pa, b, s]
                    copy_eng()(
                        AO2[:, pa, :, sg * 8:(sg + 1) * 8],
                        ps.rearrange("p (s b) -> p b s", s=8),
                    )

            def stage2m(Gsb):
                """Mixing, both passes per slot.  Each matmul streams the
                s-PAIR (N=512, half of it discarded) purely to keep the PE
                duty cycle high enough that the HAM clock stays at 2.4 GHz;
                the weight-switch overhead absorbs the extra columns."""
                # Matmul A streams the pair (s0, s0+1) for weight G_s0,
                # matmul B the pair (s1, s1+1) for G_s1 — both matmuls'
                # GOOD output lands at sl=0, so one copy per tile collects
                # both with s-pair (4-byte-run) writes.
                X2 = B2.rearrange("p (pa b s) -> p pa b s", pa=2, s=128)
                for t in range(64):             # 2 slots per PSUM tile
                    s0, s1 = 2 * t, 2 * t + 1
                    ps = pspool.tile([128, 1024], F32, tag="ps")
                    nc.tensor.matmul(
                        ps[:, 0:512],
                        Gsb[:, s0 * 128:(s0 + 1) * 128],
                        X2[:, :, :, s0:s0 + 2], start=True, stop=True,
                    )
                    blo = s1 if t < 63 else 126
                    nc.tensor.matmul(
                        ps[:, 512:1024],
                        Gsb[:, s1 * 128:(s1 + 1) * 128],
                        X2[:, :, :, blo:blo + 2], start=True, stop=True,
                    )
                    psv = ps.rearrange("p (g pa b sl) -> p pa b g sl",
                                       g=2, pa=2, sl=2)
                    if t < 63:
                        # good halves: A at sl=0 (s0), B at sl=0 (s1)
                        copy_eng()(
                            AO2[:, :, :, s0:s0 + 2], psv[:, :, :, :, 0],
                        )
                    else:
                        copy_eng()(AO2[:, :, :, 126], psv[:, :, :, 0, 0])
                        copy_eng()(AO2[:, :, :, 127], psv[:, :, :, 1, 1])

            def stage3(p):
                """IDFT with Ri as the stationary: N=512 matmuls covering 4
                output blocks each (rhs = T2v[:, :, i:i+4]), one 2KB PSUM
                zero-region per matmul so A/B accumulation pairs are legal.
                Output layout [t', (u2, b, i4)]; the host reorders."""
                T2v = B2v[:, p]                 # [s, b, m]
                for c in range(2):              # t' chunk
                    for e in range(8):          # 8 output blocks i per tile
                        ps = pspool.tile([128, 1024], F32, tag="ps",
                                         name="ps3")
                        for h in range(2):      # RiA then RiB (accumulate)
                            off = c * 128 if h == 0 else 256 + c * 128
                            for u in range(2):
                                i0 = h * 64 + e * 8 + u * 4
                                nc.tensor.matmul(
                                    ps[:, u * 512:(u + 1) * 512],
                                    Risb[:, off:off + 128],
                                    T2v[:, :, i0:i0 + 4],
                                    start=(h == 0), stop=(h == 1),
                                )
                        yt = wpool.tile([128, 1024], BF16, tag="yt",
                                        name="yt")
                        copy_eng()(yt[:, :], ps[:, :])
                        nc.gpsimd.dma_start(Y.ap()[p, c, e], yt[:, :])

            # Emission.
            # PE:   S1p0 | S1p1+T1p0 | S2a(p0) | S2a(p1) | S2m |
            #       T2p1-batch | S3p1 | S3p0
            # xbar: T1p1 chunks (under S1p1/S2a) ... T2p0 (under T2p1/S3p1)
            load_xk(0, nc.sync)
            load_xk(1, nc.sync)
            Gsb = load_g()
            stage1(0)
            # perm1(p0): PE transposes interleaved; perm1(p1): xbar chunks
            # for b<96 + a PE coda for b 96..127 (keeps the PE fed through
            # the xbar tail so the HAM clock stays warm)
            stage1(1, inter=perm_steps(Ao1[:, 0], B2v[:, 0]), xbar_perm1=True)
            for fn in perm_steps(Ao1[:, 1], B2v[:, 1])[12:16]:
                fn()
            stage2m(Gsb)
            # perm2(p0): xbar for b<96 + PE coda; perm2(p1) on the PE
            for c in range(3):
                nc.sync.dma_start(
                    B2v[:, 0, c * 32:(c + 1) * 32, :],
                    A[:, c * 4096:(c + 1) * 4096], transpose=True,
                )
            for fn in perm_steps(AO2[:, 1], B2v[:, 1]):
                fn()
            for fn in perm_steps(AO2[:, 0], B2v[:, 0])[12:16]:
                fn()
            stage3(1)
            stage3(0)
    nc.compile()
    return nc


def _get_nc():
    if "nc" not in _NC_CACHE:
        _NC_CACHE["nc"] = _build_nc()
    return _NC_CACHE["nc"]


def run(x, W_real, W_imag, trace=False):
    x = np.asarray(x, dtype=np.float32)
    Wr = np.asarray(W_real, dtype=np.float32)
    Wi = np.asarray(W_imag, dtype=np.float32)

    nc = _get_nc()
    R, Ri = _build_consts()
    G = _build_g(Wr, Wi)
    Rflat = np.ascontiguousarray(R.reshape(128, 512))
    Iden = np.eye(128, dtype=NPBF16)

    in_maps = []
    for c in range(NCORES):
        xc = x[c * BC:(c + 1) * BC]                       # [256, 16384]
        # -> [pass, tl, b, kt, j]: x[pass*128+b, j, kt*128+tl]
        xcp = xc.reshape(NPASS, PB, KIN, 2, 128).transpose(0, 4, 1, 3, 2)
        in_maps.append({
            "xP": np.ascontiguousarray(xcp).astype(NPBF16),
            "G": G, "R": Rflat, "Ri": Ri, "Iden": Iden,
        })
    res = run_bass_kernel_spmd(
        nc, in_maps, core_ids=list(range(NCORES)), trace=trace
    )
    outs = []
    for r in res.results:
        yc = r["Y"].astype(np.float32)        # [pass, c, e, t', (u, b, i4)]
        yc = yc.reshape(NPASS, 2, 8, 128, 2, PB, 4)
        # i = e*8 + u*4 + i4; col = i*256 + c*128 + t
        yc = yc.transpose(0, 5, 2, 4, 6, 1, 3)  # [pass, b, e, u, i4, c, t]
        outs.append(yc.reshape(BC, KOUT * BLOCK))
    out = np.concatenate(outs, axis=0)
    return np.ascontiguousarray(out), res


def kernel(x, W_real, W_imag):
    out, _ = run(x, W_real, W_imag)
    return out


# revision 27
# speedup vs baseline: 1.4664x; 1.0598x over previous
"""Block-FFT circulant matmul (BlockFFTDirectPrior) as a Trainium2 Bass kernel.

Math: out = ifft( einsum('bjf,ijf->bif', fft(x_blocks), conj(W_full)) ).real
with 64x64 blocks of size 256, batch 2048.

All matmul-based (no FFT butterflies):
  stage 1: per input block j, spectrum = x_j @ R            (DFT as matmul)
  stage 2: per frequency slot s, mix blocks j -> i with a 128x128 real
           matrix G_s built from W_real/W_imag (re/im packed)
  stage 3: per output block i, time = spectrum_i @ Rinv     (IDFT as matmul)

v5: two 64KB/partition ping-pong buffers (A holds x -> stage-1 output
in place -> stage-2 output; B2 holds the transposed spectra).  The two
inter-stage permutes per pass are split between the DMA xbar (which
monopolizes the fabric, so it only runs in wire-quiet windows: perm1
pass-1 chunked under stage-1 pass-1, perm2 pass-0 paired with a
concurrent PE-transpose batch) and tensor-engine is_transpose matmuls
(perm1 pass-0 interleaved in stage-1 pass-1's warm matmul stream,
perm2 pass-1 batched while the xbar runs).  Stage 2 is merged across
both passes: one LDWEIGHTS + one N=256 matmul per frequency slot,
halving its time and keeping the PE busy-fraction high enough that the
HAM clock stays at 2.4 GHz.

Sharding: data-parallel over batch across 8 NeuronCores (256 rows each),
2 passes of 128 rows per core.  All matmul operands are bf16.
"""

import numpy as np
import ml_dtypes

import concourse.bass as bass
import concourse.mybir as mybir
from concourse import bacc
from concourse.tile import TileContext
from concourse.bass_utils import run_bass_kernel_spmd

B, KIN, KOUT, BLOCK = 2048, 64, 64, 256
NCORES = 8
BC = B // NCORES            # 256 batch rows per core
NPASS = 2
PB = BC // NPASS            # 128 batch rows per pass

F32 = mybir.dt.float32
BF16 = mybir.dt.bfloat16
NPBF16 = ml_dtypes.bfloat16

_NC_CACHE = {}


def _build_consts():
    """DFT / inverse-DFT matrices, bf16, kernel layouts."""
    t = np.arange(BLOCK)
    f = np.arange(128)
    ang = 2.0 * np.pi * np.outer(t, f) / BLOCK          # [t, f]
    RA = np.cos(ang)                                    # re f=0..127
    RB = -np.sin(ang)                                   # im f=1..127
    RB[:, 0] = np.cos(np.pi * t)                        # re f=128 in col 0
    R = np.zeros((128, 4, 128), dtype=NPBF16)           # [t128, (h,kt), s]
    for h in range(2):
        M = RA if h == 0 else RB
        for kt in range(2):
            R[:, h * 2 + kt, :] = M[kt * 128:(kt + 1) * 128, :].astype(NPBF16)

    s = np.arange(128)
    tp = np.arange(BLOCK)
    angi = 2.0 * np.pi * np.outer(s, tp) / BLOCK        # [s, t']
    w = np.full((128, 1), 2.0 / BLOCK)
    w[0] = 1.0 / BLOCK
    RiA = w * np.cos(angi)
    RiB = -(2.0 / BLOCK) * np.sin(angi)
    RiB[0, :] = (1.0 / BLOCK) * np.cos(np.pi * tp)      # Nyquist (real) term
    Ri = np.concatenate([RiA, RiB], axis=1).astype(NPBF16)  # [128, 512]
    return R, Ri


def _build_g(Wr, Wi):
    """Stage-2 mixing matrices, layout [k=(h*64+j), s, m=(re_i|im_i)], bf16."""
    G = np.zeros((128, 128, 128), dtype=np.float32)     # [s, k, m]
    G[0, :64, :64] = Wr[:, :, 0].T
    G[0, 64:, 64:] = Wr[:, :, 128].T
    WrT = np.transpose(Wr, (2, 1, 0))                   # [f, j, i]
    WiT = np.transpose(Wi, (2, 1, 0))
    G[1:, :64, :64] = WrT[1:128]
    G[1:, :64, 64:] = -WiT[1:128]
    G[1:, 64:, :64] = WiT[1:128]
    G[1:, 64:, 64:] = WrT[1:128]
    return np.ascontiguousarray(G.transpose(1, 0, 2)).astype(NPBF16)


def _build_nc():
    nc = bacc.Bacc("TRN2", target_bir_lowering=False, debug=False)
    # xP layout [pass, tl(128), b(128), kt(2), j(64)]
    xP = nc.dram_tensor("xP", [NPASS, 128, PB, 2, KIN], BF16,
                        kind="ExternalInput")
    Gt = nc.dram_tensor("G", [128, 128, 128], BF16, kind="ExternalInput")
    Rt = nc.dram_tensor("R", [128, 4 * 128], BF16, kind="ExternalInput")
    Rit = nc.dram_tensor("Ri", [128, 512], BF16, kind="ExternalInput")
    It = nc.dram_tensor("Iden", [128, 128], BF16, kind="ExternalInput")
    # Y layout [pass, t'-chunk(2), i-oct(8), t'(128), (u2, b, i4)];
    # the host untangles this back to [b, i, t'] for free.
    Y = nc.dram_tensor("Y", [NPASS, 2, 8, 128, 8 * PB], BF16,
                       kind="ExternalOutput")

    nck = [0]

    def copy_eng():
        nck[0] += 1
        return nc.vector.tensor_copy if nck[0] % 2 == 0 else nc.scalar.copy

    with TileContext(nc) as tc:
        with (
            tc.tile_pool(name="const", bufs=1) as cpool,
            tc.tile_pool(name="big", bufs=1) as bigpool,
            tc.tile_pool(name="work", bufs=4) as wpool,
            tc.tile_pool(name="ps", bufs=3, space="PSUM") as pspool,
            tc.tile_pool(name="pt", bufs=2, space="PSUM") as ptpool,
        ):
            # constants: R/Ri/Iden on scalar queue, G on gpsimd
            Rsb = cpool.tile([128, 4 * 128], BF16)
            nc.scalar.dma_start(Rsb[:, :], Rt.ap()[:, :])
            Risb = cpool.tile([128, 512], BF16)
            nc.scalar.dma_start(Risb[:, :], Rit.ap()[:, :])
            Iden = cpool.tile([128, 128], BF16)
            nc.scalar.dma_start(Iden[:, :], It.ap()[:, :])

            # two 64KB ping-pong buffers
            # A:  xk(p0)|xk(p1) -> out1(p0)|out1(p1) in place -> O2(p0)|O2(p1)
            # B2: X2(p0)|X2(p1) -> T2(p0)|T2(p1)
            A = bigpool.tile([128, 2 * 128 * 128], BF16, tag="A", name="A")
            B2 = bigpool.tile([128, 2 * 128 * 128], BF16, tag="B2", name="B2")
            HALF = 128 * 128
            # views
            Axk = A.rearrange("p (pa b kt j) -> p pa b kt j", pa=2, kt=2, j=KIN)
            Ao1 = A.rearrange("p (pa b hj) -> p pa b hj", pa=2, hj=128)
            AO2 = A.rearrange("p (pa b s) -> p pa b s", pa=2, s=128)
            B2v = B2.rearrange("p (pa b s) -> p pa b s", pa=2, s=128)

            def load_xk(p, eng):
                for q in range(4):
                    eng.dma_start(
                        Axk[:, p, q * 32:(q + 1) * 32, :, :],
                        xP.ap()[p, :, q * 32:(q + 1) * 32],
                    )

            # G load on gpsimd, s-ordered so stage 2 can consume
            # progressively (col-chunk c covers s = 16c..16c+16).
            def load_g():
                Gsb = cpool.tile([128, 128 * 128], BF16)
                for q in range(8):
                    nc.gpsimd.dma_start(
                        Gsb[:, q * 2048:(q + 1) * 2048],
                        Gt.ap()[:, q * 16:(q + 1) * 16],
                    )
                return Gsb

            # PE-transpose permute steps: src [p, b, c128] -> dst [c128, b, p]
            # 8 transposes + 1 evict per step (one PSUM bank, bf16).
            def perm_steps(src_v, dst_v):
                def step(b8):
                    pt = ptpool.tile([128, 1024], BF16, tag="pt")
                    for o in range(8):
                        b = b8 * 8 + o
                        nc.tensor.transpose(
                            pt[:, o * 128:(o + 1) * 128],
                            src_v[:, b, :], Iden[:, :],
                        )
                    copy_eng()(
                        dst_v[:, b8 * 8:(b8 + 1) * 8, :],
                        pt.rearrange("p (b c) -> p b c", b=8),
                    )
                return [lambda b8=b8: step(b8) for b8 in range(16)]

            def stage1(p, inter=(), xbar_perm1=False):
                """DFT pass p, in place in A's half p.  Optionally fires
                perm1(p) xbar chunks (to B2) as b-ranges complete."""
                it = iter(inter)
                for g2 in range(8):             # 16-batch groups
                    pss = []
                    for h in range(2):
                        ps = pspool.tile([128, 1024], F32, tag="ps")
                        for q in range(2):
                            g = g2 * 2 + q      # 8-batch chunk
                            nc.tensor.matmul(
                                ps[:, q * 512:(q + 1) * 512],
                                Rsb[:, (h * 2) * 128:(h * 2 + 1) * 128],
                                Axk[:, p, g * 8:(g + 1) * 8, 0, :],
                                start=True, stop=False,
                            )
                            nc.tensor.matmul(
                                ps[:, q * 512:(q + 1) * 512],
                                Rsb[:, (h * 2 + 1) * 128:(h * 2 + 2) * 128],
                                Axk[:, p, g * 8:(g + 1) * 8, 1, :],
                                start=False, stop=True,
                            )
                        pss.append(ps)
                    # both h read the window before the copies overwrite it
                    for h in range(2):
                        # ps [s, (b16, j64)] -> out1 [s, b, h*64+j]
                        copy_eng()(
                            Ao1[:, p, g2 * 16:(g2 + 1) * 16,
                                h * 64:(h + 1) * 64],
                            pss[h].rearrange("p (b j) -> p b j", b=16),
                        )
                    for fn in it:               # interleaved perm steps
                        fn()
                        break
                    for fn in it:
                        fn()
                        break
                    if xbar_perm1 and g2 < 6:
                        nc.sync.dma_start(
                            B2v[:, p, g2 * 16:(g2 + 1) * 16, :],
                            A[:, p * HALF + g2 * 2048:p * HALF
                              + (g2 + 1) * 2048],
                            transpose=True,
                        )

            def stage2a(Gsb, pa, sg0, sg1):
                """Mixing for one pass only, slots s in [8*sg0, 8*sg1) —
                used to fill the pipeline while the other pass's perm1
                finishes (keeps the HAM clock warm)."""
                X2 = B2.rearrange("p (pa b s) -> p pa b s", pa=2, s=128)
                for sg in range(sg0, sg1):      # 8 slots per PSUM tile
                    ps = pspool.tile([128, 1024], F32, tag="ps")
                    for u in range(8):
                        s = sg * 8 + u
                        nc.tensor.matmul(
                            ps[:, u * 128:(u + 1) * 128],
                            Gsb[:, s * 128:(s + 1) * 128],
                            X2[:, pa, :, s],
                            start=True, stop=True,
                        )
                    # ps [m, (s8, b)] -> O2 [m, pa, b, s]
                    copy_eng()(
                        AO2[:, pa, :, sg * 8:(sg + 1) * 8],
                        ps.rearrange("p (s b) -> p b s", s=8),
                    )

            def stage2m(Gsb):
                """Mixing, both passes per slot: one LDW + one N=256 MM,
                slots s = 32..127."""
                X2 = B2.rearrange("p (pa b s) -> p pa b s", pa=2, s=128)
                for sg in range(8, 32):         # 4 slots per PSUM tile
                    ps = pspool.tile([128, 1024], F32, tag="ps")
                    for u in range(4):
                        s = sg * 4 + u
                        nc.tensor.matmul(
                            ps[:, u * 256:(u + 1) * 256],
                            Gsb[:, s * 128:(s + 1) * 128],
                            X2[:, :, :, s],
                            start=True, stop=True,
                        )
                    # ps [m, (s4, pa, b)] -> O2 [m, pa, b, s]; split the
                    # evict across both engines so they run concurrently
                    nc.vector.tensor_copy(
                        AO2[:, :, :, sg * 4:sg * 4 + 2],
                        ps[:, 0:512].rearrange(
                            "p (s pa b) -> p pa b s", s=2, pa=2),
                    )
                    nc.scalar.copy(
                        AO2[:, :, :, sg * 4 + 2:sg * 4 + 4],
                        ps[:, 512:1024].rearrange(
                            "p (s pa b) -> p pa b s", s=2, pa=2),
                    )

            def stage3(p):
                """IDFT with Ri as the stationary: N=512 matmuls covering 4
                output blocks each (rhs = T2v[:, :, i:i+4]), one 2KB PSUM
                zero-region per matmul so A/B accumulation pairs are legal.
                Output layout [t', (u2, b, i4)]; the host reorders."""
                T2v = B2v[:, p]                 # [s, b, m]
                for c in range(2):              # t' chunk
                    for e in range(8):          # 8 output blocks i per tile
                        ps = pspool.tile([128, 1024], F32, tag="ps",
                                         name="ps3")
                        for h in range(2):      # RiA then RiB (accumulate)
                            off = c * 128 if h == 0 else 256 + c * 128
                            for u in range(2):
                                i0 = h * 64 + e * 8 + u * 4
                                nc.tensor.matmul(
                                    ps[:, u * 512:(u + 1) * 512],
                                    Risb[:, off:off + 128],
                                    T2v[:, :, i0:i0 + 4],
                                    start=(h == 0), stop=(h == 1),
                                )
                        yt = wpool.tile([128, 1024], BF16, tag="yt",
                                        name="yt")
                        copy_eng()(yt[:, :], ps[:, :])
                        nc.gpsimd.dma_start(Y.ap()[p, c, e], yt[:, :])

            # Emission.
            # PE:   S1p0 | S1p1+T1p0 | S2a(p0) | S2a(p1) | S2m |
            #       T2p1-batch | S3p1 | S3p0
            # xbar: T1p1 chunks (under S1p1/S2a) ... T2p0 (under T2p1/S3p1)
            load_xk(0, nc.sync)
            load_xk(1, nc.sync)
            Gsb = load_g()
            stage1(0)
            # perm1(p0): PE transposes interleaved; perm1(p1): xbar chunks
            # for b<96 + a PE coda for b 96..127 (keeps the PE fed through
            # the xbar tail so the HAM clock stays warm)
            stage1(1, inter=perm_steps(Ao1[:, 0], B2v[:, 0]), xbar_perm1=True)
            for fn in perm_steps(Ao1[:, 1], B2v[:, 1])[12:16]:
                fn()
            stage2a(Gsb, 0, 0, 4)       # p0-only slots: fills the xbar tail
            stage2a(Gsb, 1, 0, 4)
            stage2m(Gsb)
            # perm2(p0): xbar for b<96 + PE coda; perm2(p1) on the PE
            for c in range(3):
                nc.sync.dma_start(
                    B2v[:, 0, c * 32:(c + 1) * 32, :],
                    A[:, c * 4096:(c + 1) * 4096], transpose=True,
                )
            for fn in perm_steps(AO2[:, 1], B2v[:, 1]):
                fn()
            for fn in perm_steps(AO2[:, 0], B2v[:, 0])[12:16]:
                fn()
            stage3(1)
            stage3(0)
    nc.compile()
    return nc


def _get_nc():
    if "nc" not in _NC_CACHE:
        _NC_CACHE["nc"] = _build_nc()
    return _NC_CACHE["nc"]


def run(x, W_real, W_imag, trace=False):
    x = np.asarray(x, dtype=np.float32)
    Wr = np.asarray(W_real, dtype=np.float32)
    Wi = np.asarray(W_imag, dtype=np.float32)

    nc = _get_nc()
    R, Ri = _build_consts()
    G = _build_g(Wr, Wi)
    Rflat = np.ascontiguousarray(R.reshape(128, 512))
    Iden = np.eye(128, dtype=NPBF16)

    in_maps = []
    for c in range(NCORES):
        xc = x[c * BC:(c + 1) * BC]                       # [256, 16384]
        # -> [pass, tl, b, kt, j]: x[pass*128+b, j, kt*128+tl]
        xcp = xc.reshape(NPASS, PB, KIN, 2, 128).transpose(0, 4, 1, 3, 2)
        in_maps.append({
            "xP": np.ascontiguousarray(xcp).astype(NPBF16),
            "G": G, "R": Rflat, "Ri": Ri, "Iden": Iden,
        })
    res = run_bass_kernel_spmd(
        nc, in_maps, core_ids=list(range(NCORES)), trace=trace
    )
    outs = []
    for r in res.results:
        yc = r["Y"].astype(np.float32)        # [pass, c, e, t', (u, b, i4)]
        yc = yc.reshape(NPASS, 2, 8, 128, 2, PB, 4)
        # i = e*8 + u*4 + i4; col = i*256 + c*128 + t
        yc = yc.transpose(0, 5, 2, 4, 6, 1, 3)  # [pass, b, e, u, i4, c, t]
        outs.append(yc.reshape(BC, KOUT * BLOCK))
    out = np.concatenate(outs, axis=0)
    return np.ascontiguousarray(out), res


def kernel(x, W_real, W_imag):
    out, _ = run(x, W_real, W_imag)
    return out


# revision 31
# speedup vs baseline: 1.5014x; 1.0238x over previous
"""Block-FFT circulant matmul (BlockFFTDirectPrior) as a Trainium2 Bass kernel.

Math: out = ifft( einsum('bjf,ijf->bif', fft(x_blocks), conj(W_full)) ).real
with 64x64 blocks of size 256, batch 2048.

All matmul-based (no FFT butterflies):
  stage 1: per input block j, spectrum = x_j @ R            (DFT as matmul)
  stage 2: per frequency slot s, mix blocks j -> i with a 128x128 real
           matrix G_s built from W_real/W_imag (re/im packed)
  stage 3: per output block i, time = spectrum_i @ Rinv     (IDFT as matmul)

v8: two 64KB/partition ping-pong buffers (A holds x -> stage-1 output
in place -> stage-2 output; B2 holds the transposed spectra).  The two
inter-stage permutes per pass are split between the DMA xbar (which
monopolizes the DMA fabric and conservatively serializes against later
readers of its dst tile, so each xbar perm covers only b<96 and a PE
coda finishes b 96..127 to keep the PE fed through the xbar tail) and
tensor-engine is_transpose matmuls (perm1 pass-0 interleaved in stage-1
pass-1's matmul stream; perm2 pass-1 batched while the perm2 pass-0
xbar runs).  Stage 2 runs single-pass slots for s<32 (filling the
perm1 xbar tail) then merged both-pass slots (one LDWEIGHTS + one
N=256 matmul per frequency).  Stage 3 uses Ri as the stationary with
N=512 matmuls covering 4 output blocks each (one PSUM zero-region per
matmul), storing [t', (i, b)]-layout that the host re-permutes for
free.  Stage-3 pass 1 runs before pass 0 since its input comes from
the PE-side perm, not the xbar.

Sharding: data-parallel over batch across 8 NeuronCores (256 rows each),
2 passes of 128 rows per core.  All matmul operands are bf16.
"""

import numpy as np
import ml_dtypes

import concourse.bass as bass
import concourse.mybir as mybir
from concourse import bacc
from concourse.tile import TileContext
from concourse.bass_utils import run_bass_kernel_spmd

B, KIN, KOUT, BLOCK = 2048, 64, 64, 256
NCORES = 8
BC = B // NCORES            # 256 batch rows per core
NPASS = 2
PB = BC // NPASS            # 128 batch rows per pass

F32 = mybir.dt.float32
BF16 = mybir.dt.bfloat16
NPBF16 = ml_dtypes.bfloat16

_NC_CACHE = {}


def _build_consts():
    """DFT / inverse-DFT matrices, bf16, kernel layouts."""
    t = np.arange(BLOCK)
    f = np.arange(128)
    ang = 2.0 * np.pi * np.outer(t, f) / BLOCK          # [t, f]
    RA = np.cos(ang)                                    # re f=0..127
    RB = -np.sin(ang)                                   # im f=1..127
    RB[:, 0] = np.cos(np.pi * t)                        # re f=128 in col 0
    R = np.zeros((128, 4, 128), dtype=NPBF16)           # [t128, (h,kt), s]
    for h in range(2):
        M = RA if h == 0 else RB
        for kt in range(2):
            R[:, h * 2 + kt, :] = M[kt * 128:(kt + 1) * 128, :].astype(NPBF16)

    s = np.arange(128)
    tp = np.arange(BLOCK)
    angi = 2.0 * np.pi * np.outer(s, tp) / BLOCK        # [s, t']
    w = np.full((128, 1), 2.0 / BLOCK)
    w[0] = 1.0 / BLOCK
    RiA = w * np.cos(angi)
    RiB = -(2.0 / BLOCK) * np.sin(angi)
    RiB[0, :] = (1.0 / BLOCK) * np.cos(np.pi * tp)      # Nyquist (real) term
    Ri = np.concatenate([RiA, RiB], axis=1).astype(NPBF16)  # [128, 512]
    return R, Ri


def _build_g(Wr, Wi):
    """Stage-2 mixing matrices, layout [k=(h*64+j), s, m=(re_i|im_i)], bf16."""
    G = np.zeros((128, 128, 128), dtype=np.float32)     # [s, k, m]
    G[0, :64, :64] = Wr[:, :, 0].T
    G[0, 64:, 64:] = Wr[:, :, 128].T
    WrT = np.transpose(Wr, (2, 1, 0))                   # [f, j, i]
    WiT = np.transpose(Wi, (2, 1, 0))
    G[1:, :64, :64] = WrT[1:128]
    G[1:, :64, 64:] = -WiT[1:128]
    G[1:, 64:, :64] = WiT[1:128]
    G[1:, 64:, 64:] = WrT[1:128]
    return np.ascontiguousarray(G.transpose(1, 0, 2)).astype(NPBF16)


def _build_nc():
    nc = bacc.Bacc("TRN2", target_bir_lowering=False, debug=False)
    # xP layout [pass, tl(128), b(128), kt(2), j(64)]
    xP = nc.dram_tensor("xP", [NPASS, 128, PB, 2, KIN], BF16,
                        kind="ExternalInput")
    Gt = nc.dram_tensor("G", [128, 128, 128], BF16, kind="ExternalInput")
    Rt = nc.dram_tensor("R", [128, 4 * 128], BF16, kind="ExternalInput")
    Rit = nc.dram_tensor("Ri", [128, 512], BF16, kind="ExternalInput")
    It = nc.dram_tensor("Iden", [128, 128], BF16, kind="ExternalInput")
    # Y layout [pass, t'-chunk(2), i-oct(8), t'(128), (u2, b, i4)];
    # the host untangles this back to [b, i, t'] for free.
    Y = nc.dram_tensor("Y", [NPASS, 2, 8, 128, 8 * PB], BF16,
                       kind="ExternalOutput")

    nck = [0]

    def copy_eng():
        nck[0] += 1
        return nc.vector.tensor_copy if nck[0] % 2 == 0 else nc.scalar.copy

    with TileContext(nc) as tc:
        with (
            tc.tile_pool(name="const", bufs=1) as cpool,
            tc.tile_pool(name="big", bufs=1) as bigpool,
            tc.tile_pool(name="work", bufs=4) as wpool,
            tc.tile_pool(name="ps", bufs=3, space="PSUM") as pspool,
            tc.tile_pool(name="pt", bufs=2, space="PSUM") as ptpool,
        ):
            # constants: R/Ri/Iden on scalar queue, G on gpsimd
            Rsb = cpool.tile([128, 4 * 128], BF16)
            nc.scalar.dma_start(Rsb[:, :], Rt.ap()[:, :])
            Risb = cpool.tile([128, 512], BF16)
            nc.scalar.dma_start(Risb[:, :], Rit.ap()[:, :])
            Iden = cpool.tile([128, 128], BF16)
            nc.scalar.dma_start(Iden[:, :], It.ap()[:, :])

            # two 64KB ping-pong buffers
            # A:  xk(p0)|xk(p1) -> out1(p0)|out1(p1) in place -> O2(p0)|O2(p1)
            # B2: X2(p0)|X2(p1) -> T2(p0)|T2(p1)
            A = bigpool.tile([128, 2 * 128 * 128], BF16, tag="A", name="A")
            B2 = bigpool.tile([128, 2 * 128 * 128], BF16, tag="B2", name="B2")
            HALF = 128 * 128
            # views
            Axk = A.rearrange("p (pa b kt j) -> p pa b kt j", pa=2, kt=2, j=KIN)
            Ao1 = A.rearrange("p (pa b hj) -> p pa b hj", pa=2, hj=128)
            AO2 = A.rearrange("p (pa b s) -> p pa b s", pa=2, s=128)
            B2v = B2.rearrange("p (pa b s) -> p pa b s", pa=2, s=128)

            def load_xk(p, eng):
                for q in range(4):
                    eng.dma_start(
                        Axk[:, p, q * 32:(q + 1) * 32, :, :],
                        xP.ap()[p, :, q * 32:(q + 1) * 32],
                    )

            # G load, s-ordered so stage 2 can consume progressively
            # (col-chunk c covers s = 16c..16c+16).  Issued on sync AFTER
            # the x loads so the x wire gets the full fabric rate; G is not
            # needed until stage 2 and still lands in time behind the
            # perm1 xbar chunks.
            def load_g(eng):
                Gsb = cpool.tile([128, 128 * 128], BF16)
                for q in range(8):
                    eng.dma_start(
                        Gsb[:, q * 2048:(q + 1) * 2048],
                        Gt.ap()[:, q * 16:(q + 1) * 16],
                    )
                return Gsb

            # PE-transpose permute steps: src [p, b, c128] -> dst [c128, b, p]
            # 8 transposes + 1 evict per step (one PSUM bank, bf16).
            def perm_steps(src_v, dst_v):
                def step(b8):
                    pt = ptpool.tile([128, 1024], BF16, tag="pt")
                    for o in range(8):
                        b = b8 * 8 + o
                        nc.tensor.transpose(
                            pt[:, o * 128:(o + 1) * 128],
                            src_v[:, b, :], Iden[:, :],
                        )
                    copy_eng()(
                        dst_v[:, b8 * 8:(b8 + 1) * 8, :],
                        pt.rearrange("p (b c) -> p b c", b=8),
                    )
                return [lambda b8=b8: step(b8) for b8 in range(16)]

            def stage1(p, inter=(), xbar_perm1=False):
                """DFT pass p, in place in A's half p.  Optionally fires
                perm1(p) xbar chunks (to B2) as b-ranges complete."""
                it = iter(inter)
                for g2 in range(8):             # 16-batch groups
                    pss = []
                    for h in range(2):
                        ps = pspool.tile([128, 1024], F32, tag="ps")
                        for q in range(2):
                            g = g2 * 2 + q      # 8-batch chunk
                            nc.tensor.matmul(
                                ps[:, q * 512:(q + 1) * 512],
                                Rsb[:, (h * 2) * 128:(h * 2 + 1) * 128],
                                Axk[:, p, g * 8:(g + 1) * 8, 0, :],
                                start=True, stop=False,
                            )
                            nc.tensor.matmul(
                                ps[:, q * 512:(q + 1) * 512],
                                Rsb[:, (h * 2 + 1) * 128:(h * 2 + 2) * 128],
                                Axk[:, p, g * 8:(g + 1) * 8, 1, :],
                                start=False, stop=True,
                            )
                        pss.append(ps)
                    # both h read the window before the copies overwrite it
                    for h in range(2):
                        # ps [s, (b16, j64)] -> out1 [s, b, h*64+j]
                        copy_eng()(
                            Ao1[:, p, g2 * 16:(g2 + 1) * 16,
                                h * 64:(h + 1) * 64],
                            pss[h].rearrange("p (b j) -> p b j", b=16),
                        )
                    for fn in it:               # interleaved perm steps
                        fn()
                        break
                    for fn in it:
                        fn()
                        break
                    if xbar_perm1 and g2 < 6:
                        nc.sync.dma_start(
                            B2v[:, p, g2 * 16:(g2 + 1) * 16, :],
                            A[:, p * HALF + g2 * 2048:p * HALF
                              + (g2 + 1) * 2048],
                            transpose=True,
                        )

            def stage2a(Gsb, pa, sg0, sg1):
                """Mixing for one pass only, slots s in [8*sg0, 8*sg1) —
                used to fill the pipeline while the other pass's perm1
                finishes (keeps the HAM clock warm)."""
                X2 = B2.rearrange("p (pa b s) -> p pa b s", pa=2, s=128)
                for sg in range(sg0, sg1):      # 8 slots per PSUM tile
                    ps = pspool.tile([128, 1024], F32, tag="ps")
                    for u in range(8):
                        s = sg * 8 + u
                        nc.tensor.matmul(
                            ps[:, u * 128:(u + 1) * 128],
                            Gsb[:, s * 128:(s + 1) * 128],
                            X2[:, pa, :, s],
                            start=True, stop=True,
                        )
                    # ps [m, (s8, b)] -> O2 [m, pa, b, s]
                    copy_eng()(
                        AO2[:, pa, :, sg * 8:(sg + 1) * 8],
                        ps.rearrange("p (s b) -> p b s", s=8),
                    )

            def stage2m(Gsb):
                """Mixing, both passes per slot: one LDW + one N=256 MM,
                slots s = 32..127."""
                X2 = B2.rearrange("p (pa b s) -> p pa b s", pa=2, s=128)
                for sg in range(8, 32):         # 4 slots per PSUM tile
                    ps = pspool.tile([128, 1024], F32, tag="ps")
                    for u in range(4):
                        s = sg * 4 + u
                        nc.tensor.matmul(
                            ps[:, u * 256:(u + 1) * 256],
                            Gsb[:, s * 128:(s + 1) * 128],
                            X2[:, :, :, s],
                            start=True, stop=True,
                        )
                    # ps [m, (s4, pa, b)] -> O2 [m, pa, b, s]; split the
                    # evict across both engines so they run concurrently
                    nc.vector.tensor_copy(
                        AO2[:, :, :, sg * 4:sg * 4 + 2],
                        ps[:, 0:512].rearrange(
                            "p (s pa b) -> p pa b s", s=2, pa=2),
                    )
                    nc.scalar.copy(
                        AO2[:, :, :, sg * 4 + 2:sg * 4 + 4],
                        ps[:, 512:1024].rearrange(
                            "p (s pa b) -> p pa b s", s=2, pa=2),
                    )

            def stage3(p):
                """IDFT with Ri as the stationary: N=512 matmuls covering 4
                output blocks each (rhs = T2v[:, :, i:i+4]), one 2KB PSUM
                zero-region per matmul so A/B accumulation pairs are legal.
                Output layout [t', (u2, b, i4)]; the host reorders."""
                T2v = B2v[:, p]                 # [s, b, m]
                for c in range(2):              # t' chunk
                    for e in range(8):          # 8 output blocks i per tile
                        ps = pspool.tile([128, 1024], F32, tag="ps",
                                         name="ps3")
                        for h in range(2):      # RiA then RiB (accumulate)
                            off = c * 128 if h == 0 else 256 + c * 128
                            for u in range(2):
                                i0 = h * 64 + e * 8 + u * 4
                                nc.tensor.matmul(
                                    ps[:, u * 512:(u + 1) * 512],
                                    Risb[:, off:off + 128],
                                    T2v[:, :, i0:i0 + 4],
                                    start=(h == 0), stop=(h == 1),
                                )
                        yt = wpool.tile([128, 1024], BF16, tag="yt",
                                        name="yt")
                        nc.vector.tensor_copy(yt[:, 0:512], ps[:, 0:512])
                        nc.scalar.copy(yt[:, 512:1024], ps[:, 512:1024])
                        nc.gpsimd.dma_start(Y.ap()[p, c, e], yt[:, :])

            # Emission.
            # PE:   S1p0 | S1p1+T1p0 | S2a(p0) | S2a(p1) | S2m |
            #       T2p1-batch | S3p1 | S3p0
            # xbar: T1p1 chunks (under S1p1/S2a) ... T2p0 (under T2p1/S3p1)
            load_xk(0, nc.sync)
            load_xk(1, nc.sync)
            Gsb = load_g(nc.sync)
            stage1(0)
            # perm1(p0): PE transposes interleaved; perm1(p1): xbar chunks
            # for b<96 + a PE coda for b 96..127 (keeps the PE fed through
            # the xbar tail so the HAM clock stays warm)
            stage1(1, inter=perm_steps(Ao1[:, 0], B2v[:, 0]), xbar_perm1=True)
            for fn in perm_steps(Ao1[:, 1], B2v[:, 1])[12:16]:
                fn()
            stage2a(Gsb, 0, 0, 4)       # p0-only slots: fills the xbar tail
            stage2a(Gsb, 1, 0, 4)
            stage2m(Gsb)
            # perm2(p0): xbar for b<96 + PE coda; perm2(p1) on the PE
            for c in range(3):
                nc.sync.dma_start(
                    B2v[:, 0, c * 32:(c + 1) * 32, :],
                    A[:, c * 4096:(c + 1) * 4096], transpose=True,
                )
            for fn in perm_steps(AO2[:, 1], B2v[:, 1]):
                fn()
            for fn in perm_steps(AO2[:, 0], B2v[:, 0])[12:16]:
                fn()
            stage3(1)
            stage3(0)
    nc.compile()
    return nc


def _get_nc():
    if "nc" not in _NC_CACHE:
        _NC_CACHE["nc"] = _build_nc()
    return _NC_CACHE["nc"]


def run(x, W_real, W_imag, trace=False):
    x = np.asarray(x, dtype=np.float32)
    Wr = np.asarray(W_real, dtype=np.float32)
    Wi = np.asarray(W_imag, dtype=np.float32)

    nc = _get_nc()
    R, Ri = _build_consts()
    G = _build_g(Wr, Wi)
    Rflat = np.ascontiguousarray(R.reshape(128, 512))
    Iden = np.eye(128, dtype=NPBF16)

    in_maps = []
    for c in range(NCORES):
        xc = x[c * BC:(c + 1) * BC]                       # [256, 16384]
        # -> [pass, tl, b, kt, j]: x[pass*128+b, j, kt*128+tl]
        xcp = xc.reshape(NPASS, PB, KIN, 2, 128).transpose(0, 4, 1, 3, 2)
        in_maps.append({
            "xP": np.ascontiguousarray(xcp).astype(NPBF16),
            "G": G, "R": Rflat, "Ri": Ri, "Iden": Iden,
        })
    res = run_bass_kernel_spmd(
        nc, in_maps, core_ids=list(range(NCORES)), trace=trace
    )
    outs = []
    for r in res.results:
        yc = r["Y"].astype(np.float32)        # [pass, c, e, t', (u, b, i4)]
        yc = yc.reshape(NPASS, 2, 8, 128, 2, PB, 4)
        # i = e*8 + u*4 + i4; col = i*256 + c*128 + t
        yc = yc.transpose(0, 5, 2, 4, 6, 1, 3)  # [pass, b, e, u, i4, c, t]
        outs.append(yc.reshape(BC, KOUT * BLOCK))
    out = np.concatenate(outs, axis=0)
    return np.ascontiguousarray(out), res


def kernel(x, W_real, W_imag):
    out, _ = run(x, W_real, W_imag)
    return out


# revision 33
# speedup vs baseline: 1.5219x; 1.0137x over previous
"""Block-FFT circulant matmul (BlockFFTDirectPrior) as a Trainium2 Bass kernel.

Math: out = ifft( einsum('bjf,ijf->bif', fft(x_blocks), conj(W_full)) ).real
with 64x64 blocks of size 256, batch 2048.

All matmul-based (no FFT butterflies):
  stage 1: per input block j, spectrum = x_j @ R            (DFT as matmul)
  stage 2: per frequency slot s, mix blocks j -> i with a 128x128 real
           matrix G_s built from W_real/W_imag (re/im packed)
  stage 3: per output block i, time = spectrum_i @ Rinv     (IDFT as matmul)

v8: two 64KB/partition ping-pong buffers (A holds x -> stage-1 output
in place -> stage-2 output; B2 holds the transposed spectra).  The two
inter-stage permutes per pass are split between the DMA xbar (which
monopolizes the DMA fabric and conservatively serializes against later
readers of its dst tile, so each xbar perm covers only b<96 and a PE
coda finishes b 96..127 to keep the PE fed through the xbar tail) and
tensor-engine is_transpose matmuls (perm1 pass-0 interleaved in stage-1
pass-1's matmul stream; perm2 pass-1 batched while the perm2 pass-0
xbar runs).  Stage 2 runs single-pass slots for s<32 (filling the
perm1 xbar tail) then merged both-pass slots (one LDWEIGHTS + one
N=256 matmul per frequency).  Stage 3 uses Ri as the stationary with
N=512 matmuls covering 4 output blocks each (one PSUM zero-region per
matmul), storing [t', (i, b)]-layout that the host re-permutes for
free.  Stage-3 pass 1 runs before pass 0 since its input comes from
the PE-side perm, not the xbar.

Sharding: data-parallel over batch across 8 NeuronCores (256 rows each),
2 passes of 128 rows per core.  All matmul operands are bf16.
"""

import numpy as np
import ml_dtypes

import concourse.bass as bass
import concourse.mybir as mybir
from concourse import bacc
from concourse.tile import TileContext
from concourse.bass_utils import run_bass_kernel_spmd

B, KIN, KOUT, BLOCK = 2048, 64, 64, 256
NCORES = 8
BC = B // NCORES            # 256 batch rows per core
NPASS = 2
PB = BC // NPASS            # 128 batch rows per pass

F32 = mybir.dt.float32
BF16 = mybir.dt.bfloat16
NPBF16 = ml_dtypes.bfloat16

_NC_CACHE = {}


def _build_consts():
    """DFT / inverse-DFT matrices, bf16, kernel layouts."""
    t = np.arange(BLOCK)
    f = np.arange(128)
    ang = 2.0 * np.pi * np.outer(t, f) / BLOCK          # [t, f]
    RA = np.cos(ang)                                    # re f=0..127
    RB = -np.sin(ang)                                   # im f=1..127
    RB[:, 0] = np.cos(np.pi * t)                        # re f=128 in col 0
    R = np.zeros((128, 4, 128), dtype=NPBF16)           # [t128, (h,kt), s]
    for h in range(2):
        M = RA if h == 0 else RB
        for kt in range(2):
            R[:, h * 2 + kt, :] = M[kt * 128:(kt + 1) * 128, :].astype(NPBF16)

    s = np.arange(128)
    tp = np.arange(BLOCK)
    angi = 2.0 * np.pi * np.outer(s, tp) / BLOCK        # [s, t']
    w = np.full((128, 1), 2.0 / BLOCK)
    w[0] = 1.0 / BLOCK
    RiA = w * np.cos(angi)
    RiB = -(2.0 / BLOCK) * np.sin(angi)
    RiB[0, :] = (1.0 / BLOCK) * np.cos(np.pi * tp)      # Nyquist (real) term
    Ri = np.concatenate([RiA, RiB], axis=1).astype(NPBF16)  # [128, 512]
    return R, Ri


def _build_g(Wr, Wi):
    """Stage-2 mixing matrices, layout [k=(h*64+j), s, m=(re_i|im_i)], bf16."""
    G = np.zeros((128, 128, 128), dtype=np.float32)     # [s, k, m]
    G[0, :64, :64] = Wr[:, :, 0].T
    G[0, 64:, 64:] = Wr[:, :, 128].T
    WrT = np.transpose(Wr, (2, 1, 0))                   # [f, j, i]
    WiT = np.transpose(Wi, (2, 1, 0))
    G[1:, :64, :64] = WrT[1:128]
    G[1:, :64, 64:] = -WiT[1:128]
    G[1:, 64:, :64] = WiT[1:128]
    G[1:, 64:, 64:] = WrT[1:128]
    return np.ascontiguousarray(G.transpose(1, 0, 2)).astype(NPBF16)


def _build_nc():
    nc = bacc.Bacc("TRN2", target_bir_lowering=False, debug=False)
    # xP layout [pass, tl(128), b(128), kt(2), j(64)]
    xP = nc.dram_tensor("xP", [NPASS, 128, PB, 2, KIN], BF16,
                        kind="ExternalInput")
    Gt = nc.dram_tensor("G", [128, 128, 128], BF16, kind="ExternalInput")
    Rt = nc.dram_tensor("R", [128, 4 * 128], BF16, kind="ExternalInput")
    Rit = nc.dram_tensor("Ri", [128, 512], BF16, kind="ExternalInput")
    It = nc.dram_tensor("Iden", [128, 128], BF16, kind="ExternalInput")
    # Y layout [pass, t'-chunk(2), i-oct(8), t'(128), (u2, b, i4)];
    # the host untangles this back to [b, i, t'] for free.
    Y = nc.dram_tensor("Y", [NPASS, 2, 8, 128, 8 * PB], BF16,
                       kind="ExternalOutput")

    nck = [0]

    def copy_eng():
        nck[0] += 1
        return nc.vector.tensor_copy if nck[0] % 2 == 0 else nc.scalar.copy

    with TileContext(nc) as tc:
        with (
            tc.tile_pool(name="const", bufs=1) as cpool,
            tc.tile_pool(name="big", bufs=1) as bigpool,
            tc.tile_pool(name="work", bufs=6) as wpool,
            tc.tile_pool(name="ps", bufs=3, space="PSUM") as pspool,
            tc.tile_pool(name="pt", bufs=2, space="PSUM") as ptpool,
        ):
            # constants: R/Ri/Iden on scalar queue, G on gpsimd
            Rsb = cpool.tile([128, 4 * 128], BF16)
            nc.scalar.dma_start(Rsb[:, :], Rt.ap()[:, :])
            Risb = cpool.tile([128, 512], BF16)
            nc.scalar.dma_start(Risb[:, :], Rit.ap()[:, :])
            Iden = cpool.tile([128, 128], BF16)
            nc.scalar.dma_start(Iden[:, :], It.ap()[:, :])

            # two 64KB ping-pong buffers
            # A:  xk(p0)|xk(p1) -> out1(p0)|out1(p1) in place -> O2(p0)|O2(p1)
            # B2: X2(p0)|X2(p1) -> T2(p0)|T2(p1)
            A = bigpool.tile([128, 2 * 128 * 128], BF16, tag="A", name="A")
            B2 = bigpool.tile([128, 2 * 128 * 128], BF16, tag="B2", name="B2")
            HALF = 128 * 128
            # views
            Axk = A.rearrange("p (pa b kt j) -> p pa b kt j", pa=2, kt=2, j=KIN)
            Ao1 = A.rearrange("p (pa b hj) -> p pa b hj", pa=2, hj=128)
            AO2 = A.rearrange("p (pa b s) -> p pa b s", pa=2, s=128)
            B2v = B2.rearrange("p (pa b s) -> p pa b s", pa=2, s=128)

            def load_xk(p, eng):
                # first 16-b sliver separately so stage 1 starts sooner
                ranges = [(0, 16), (16, 32)] if p == 0 else [(0, 32)]
                ranges += [(32, 64), (64, 96), (96, 128)]
                for lo, hi in ranges:
                    eng.dma_start(
                        Axk[:, p, lo:hi, :, :],
                        xP.ap()[p, :, lo:hi],
                    )

            # G load, s-ordered so stage 2 can consume progressively
            # (col-chunk c covers s = 16c..16c+16).  Issued on sync AFTER
            # the x loads so the x wire gets the full fabric rate; G is not
            # needed until stage 2 and still lands in time behind the
            # perm1 xbar chunks.
            def load_g(eng):
                Gsb = cpool.tile([128, 128 * 128], BF16)
                for q in range(8):
                    eng.dma_start(
                        Gsb[:, q * 2048:(q + 1) * 2048],
                        Gt.ap()[:, q * 16:(q + 1) * 16],
                    )
                return Gsb

            # PE-transpose permute steps: src [p, b, c128] -> dst [c128, b, p]
            # 8 transposes + 1 evict per step (one PSUM bank, bf16).
            def perm_steps(src_v, dst_v):
                def step(b8):
                    pt = ptpool.tile([128, 1024], BF16, tag="pt")
                    for o in range(8):
                        b = b8 * 8 + o
                        nc.tensor.transpose(
                            pt[:, o * 128:(o + 1) * 128],
                            src_v[:, b, :], Iden[:, :],
                        )
                    copy_eng()(
                        dst_v[:, b8 * 8:(b8 + 1) * 8, :],
                        pt.rearrange("p (b c) -> p b c", b=8),
                    )
                return [lambda b8=b8: step(b8) for b8 in range(16)]

            def stage1(p, inter=(), xbar_perm1=False):
                """DFT pass p, in place in A's half p.  Optionally fires
                perm1(p) xbar chunks (to B2) as b-ranges complete."""
                it = iter(inter)
                for g2 in range(8):             # 16-batch groups
                    pss = []
                    for h in range(2):
                        ps = pspool.tile([128, 1024], F32, tag="ps")
                        for q in range(2):
                            g = g2 * 2 + q      # 8-batch chunk
                            nc.tensor.matmul(
                                ps[:, q * 512:(q + 1) * 512],
                                Rsb[:, (h * 2) * 128:(h * 2 + 1) * 128],
                                Axk[:, p, g * 8:(g + 1) * 8, 0, :],
                                start=True, stop=False,
                            )
                            nc.tensor.matmul(
                                ps[:, q * 512:(q + 1) * 512],
                                Rsb[:, (h * 2 + 1) * 128:(h * 2 + 2) * 128],
                                Axk[:, p, g * 8:(g + 1) * 8, 1, :],
                                start=False, stop=True,
                            )
                        pss.append(ps)
                    # both h read the window before the copies overwrite it
                    for h in range(2):
                        # ps [s, (b16, j64)] -> out1 [s, b, h*64+j]
                        copy_eng()(
                            Ao1[:, p, g2 * 16:(g2 + 1) * 16,
                                h * 64:(h + 1) * 64],
                            pss[h].rearrange("p (b j) -> p b j", b=16),
                        )
                    for fn in it:               # interleaved perm steps
                        fn()
                        break
                    for fn in it:
                        fn()
                        break
                    if xbar_perm1 and g2 < 6:
                        nc.sync.dma_start(
                            B2v[:, p, g2 * 16:(g2 + 1) * 16, :],
                            A[:, p * HALF + g2 * 2048:p * HALF
                              + (g2 + 1) * 2048],
                            transpose=True,
                        )

            def stage2a(Gsb, pa, sg0, sg1):
                """Mixing for one pass only, slots s in [8*sg0, 8*sg1) —
                used to fill the pipeline while the other pass's perm1
                finishes (keeps the HAM clock warm)."""
                X2 = B2.rearrange("p (pa b s) -> p pa b s", pa=2, s=128)
                for sg in range(sg0, sg1):      # 8 slots per PSUM tile
                    ps = pspool.tile([128, 1024], F32, tag="ps")
                    for u in range(8):
                        s = sg * 8 + u
                        nc.tensor.matmul(
                            ps[:, u * 128:(u + 1) * 128],
                            Gsb[:, s * 128:(s + 1) * 128],
                            X2[:, pa, :, s],
                            start=True, stop=True,
                        )
                    # ps [m, (s8, b)] -> O2 [m, pa, b, s]
                    copy_eng()(
                        AO2[:, pa, :, sg * 8:(sg + 1) * 8],
                        ps.rearrange("p (s b) -> p b s", s=8),
                    )

            def stage2m(Gsb):
                """Mixing, both passes per slot: one LDW + one N=256 MM,
                slots s = 32..127."""
                X2 = B2.rearrange("p (pa b s) -> p pa b s", pa=2, s=128)
                for sg in range(8, 32):         # 4 slots per PSUM tile
                    ps = pspool.tile([128, 1024], F32, tag="ps")
                    for u in range(4):
                        s = sg * 4 + u
                        nc.tensor.matmul(
                            ps[:, u * 256:(u + 1) * 256],
                            Gsb[:, s * 128:(s + 1) * 128],
                            X2[:, :, :, s],
                            start=True, stop=True,
                        )
                    # ps [m, (s4, pa, b)] -> O2 [m, pa, b, s]; split the
                    # evict across both engines so they run concurrently
                    nc.vector.tensor_copy(
                        AO2[:, :, :, sg * 4:sg * 4 + 2],
                        ps[:, 0:512].rearrange(
                            "p (s pa b) -> p pa b s", s=2, pa=2),
                    )
                    nc.scalar.copy(
                        AO2[:, :, :, sg * 4 + 2:sg * 4 + 4],
                        ps[:, 512:1024].rearrange(
                            "p (s pa b) -> p pa b s", s=2, pa=2),
                    )

            def stage3(p):
                """IDFT with Ri as the stationary: N=512 matmuls covering 4
                output blocks each (rhs = T2v[:, :, i:i+4]), one 2KB PSUM
                zero-region per matmul so A/B accumulation pairs are legal.
                Output layout [t', (u2, b, i4)]; the host reorders."""
                T2v = B2v[:, p]                 # [s, b, m]
                for c in range(2):              # t' chunk
                    for e in range(8):          # 8 output blocks i per tile
                        ps = pspool.tile([128, 1024], F32, tag="ps",
                                         name="ps3")
                        for h in range(2):      # RiA then RiB (accumulate)
                            off = c * 128 if h == 0 else 256 + c * 128
                            for u in range(2):
                                i0 = h * 64 + e * 8 + u * 4
                                nc.tensor.matmul(
                                    ps[:, u * 512:(u + 1) * 512],
                                    Risb[:, off:off + 128],
                                    T2v[:, :, i0:i0 + 4],
                                    start=(h == 0), stop=(h == 1),
                                )
                        yt = wpool.tile([128, 1024], BF16, tag="yt",
                                        name="yt")
                        nc.vector.tensor_copy(yt[:, 0:512], ps[:, 0:512])
                        nc.scalar.copy(yt[:, 512:1024], ps[:, 512:1024])
                        nc.gpsimd.dma_start(Y.ap()[p, c, e], yt[:, :])

            # Emission.
            # PE:   S1p0 | S1p1+T1p0 | S2a(p0) | S2a(p1) | S2m |
            #       T2p1-batch | S3p1 | S3p0
            # xbar: T1p1 chunks (under S1p1/S2a) ... T2p0 (under T2p1/S3p1)
            load_xk(0, nc.sync)
            load_xk(1, nc.sync)
            Gsb = load_g(nc.sync)
            stage1(0)
            # perm1(p0): PE transposes interleaved; perm1(p1): xbar chunks
            # for b<96 + a PE coda for b 96..127 (keeps the PE fed through
            # the xbar tail so the HAM clock stays warm)
            stage1(1, inter=perm_steps(Ao1[:, 0], B2v[:, 0]), xbar_perm1=True)
            for fn in perm_steps(Ao1[:, 1], B2v[:, 1])[12:16]:
                fn()
            stage2a(Gsb, 0, 0, 4)       # p0-only slots: fills the xbar tail
            stage2a(Gsb, 1, 0, 4)
            stage2m(Gsb)
            # perm2(p0): xbar for b<96 + PE coda; perm2(p1) on the PE
            for c in range(3):
                nc.sync.dma_start(
                    B2v[:, 0, c * 32:(c + 1) * 32, :],
                    A[:, c * 4096:(c + 1) * 4096], transpose=True,
                )
            for fn in perm_steps(AO2[:, 1], B2v[:, 1]):
                fn()
            for fn in perm_steps(AO2[:, 0], B2v[:, 0])[12:16]:
                fn()
            stage3(1)
            stage3(0)
    nc.compile()
    return nc


def _get_nc():
    if "nc" not in _NC_CACHE:
        _NC_CACHE["nc"] = _build_nc()
    return _NC_CACHE["nc"]


def run(x, W_real, W_imag, trace=False):
    x = np.asarray(x, dtype=np.float32)
    Wr = np.asarray(W_real, dtype=np.float32)
    Wi = np.asarray(W_imag, dtype=np.float32)

    nc = _get_nc()
    R, Ri = _build_consts()
    G = _build_g(Wr, Wi)
    Rflat = np.ascontiguousarray(R.reshape(128, 512))
    Iden = np.eye(128, dtype=NPBF16)

    in_maps = []
    for c in range(NCORES):
        xc = x[c * BC:(c + 1) * BC]                       # [256, 16384]
        # -> [pass, tl, b, kt, j]: x[pass*128+b, j, kt*128+tl]
        xcp = xc.reshape(NPASS, PB, KIN, 2, 128).transpose(0, 4, 1, 3, 2)
        in_maps.append({
            "xP": np.ascontiguousarray(xcp).astype(NPBF16),
            "G": G, "R": Rflat, "Ri": Ri, "Iden": Iden,
        })
    res = run_bass_kernel_spmd(
        nc, in_maps, core_ids=list(range(NCORES)), trace=trace
    )
    outs = []
    for r in res.results:
        yc = r["Y"].astype(np.float32)        # [pass, c, e, t', (u, b, i4)]
        yc = yc.reshape(NPASS, 2, 8, 128, 2, PB, 4)
        # i = e*8 + u*4 + i4; col = i*256 + c*128 + t
        yc = yc.transpose(0, 5, 2, 4, 6, 1, 3)  # [pass, b, e, u, i4, c, t]
        outs.append(yc.reshape(BC, KOUT * BLOCK))
    out = np.concatenate(outs, axis=0)
    return np.ascontiguousarray(out), res


def kernel(x, W_real, W_imag):
    out, _ = run(x, W_real, W_imag)
    return out
